# revision 1
# baseline (speedup 1.0000x reference)
"""Trainium2 Bass kernel for 3x3 (k=2m+1) morphological erosion (sliding-window
min) over [B, C, H, W] fp32, B=8 sharded across 8 NeuronCores (one batch per
core).

Scheme (per core, shard = one batch of C=8 channel images, 1024x1024):
  - each partition holds RPP consecutive image rows in its free dim, so the
    vertical (row) min is a free-dim shifted tensor_tensor except at the 2
    per-partition boundary rows, whose missing neighbor rows are staged into
    a small side tile "bt" via partition-shifted SBUF->SBUF DMA (DMA has no
    partition-alignment restriction; compute engines require start partition
    0/32/64/96).
  - V pass first (consumes raw boundary rows), then H pass on the V result,
    which carries 1e9-padded / edge-duplicated halo columns.
  - All mins run on DVE (fp32 tensor_tensor, 1 elem/cycle/lane): this
    toolchain's walrus rejects Pool-engine TensorTensor and DMA accum-min,
    so DVE is the only engine that can take a 2-tensor fp32 min. The Pool
    engine is used for the SBUF->SBUF boundary-row DMAs (SWDGE) and pad
    memsets instead, and loads/stores ride the SP/ACT HWDGE rings.
  - V-stage1 is extended by one row so tmp[0] / tmp[R-2] double as the
    boundary rows' first min stage: ~3.9 DVE cycles per output element vs
    the 4.0 of a plain separable 3x3 min.
  - Per-core: DVE-bound, cost model 289us; HBM traffic ~66 MB = ~184us.
  - m>1 runs as m chained passes (DRAM ping-pong) inside one NEFF.
"""

import sys

sys.path.insert(0, "/opt/trn_rl_repo")

import numpy as np

import concourse.bass as bass
import concourse.tile as tile
from concourse import bacc, mybir

PAD = 1.0e9
F32 = mybir.dt.float32
MIN = mybir.AluOpType.min

CC = 512  # column chunk width
RPP = 8  # image rows per partition

_cache = {}


def _emit_pass(nc, pools, x_d, o_d, C, H, W, cc, rpp, beng="gpsimd"):
    """Emit one full erosion pass x_d -> o_d into the open TileContext."""
    ppi = H // rpp  # partitions per image
    ips = max(1, 128 // ppi)  # images per partition-stack
    inp, bnd, vtm, vt, htm, outp = pools
    R = rpp
    PW = cc + 2  # padded tile width
    if True:
        if True:
            for s0 in range(0, C, ips):  # image stacks
                n_img = min(ips, C - s0)
                P = n_img * ppi
                for c0 in range(0, W, cc):  # column chunks
                    last = c0 + cc == W
                    t = inp.tile([128, R, PW], F32)
                    # load with 1-col halo; at the image border memset the
                    # halo col to PAD
                    wlo = max(c0 - 1, 0)
                    whi = min(c0 + cc + 1, W)
                    dlo = 1 if c0 == 0 else 0
                    for i in range(n_img):
                        src = x_d[s0 + i, :, wlo:whi].rearrange(
                            "(p r) w -> p r w", p=ppi
                        )
                        p0 = i * ppi
                        nc.sync.dma_start(
                            t[p0 : p0 + ppi, :, dlo : dlo + (whi - wlo)], src
                        )
                    if c0 == 0:
                        nc.gpsimd.memset(t[0:P, :, 0:1], PAD)
                    if last:
                        nc.gpsimd.memset(t[0:P, :, PW - 1 : PW], PAD)

                    # boundary-row side tile: bt[p,0] = row below this
                    # partition's block (in[p+1] row 0), bt[p,1] = row above
                    # (in[p-1] row R-1); at image edges duplicate the edge
                    # row itself (min-idempotent clamp).
                    bt = bnd.tile([128, 2, PW], F32)
                    be = getattr(nc, beng)
                    for i in range(n_img):
                        p0 = i * ppi
                        pe = p0 + ppi - 1  # last partition of this image
                        be.dma_start(
                            bt[p0:pe, 0:1, :], t[p0 + 1 : pe + 1, 0:1, :]
                        )
                        be.dma_start(
                            bt[pe : pe + 1, 0:1, :], t[pe : pe + 1, R - 1 : R, :]
                        )
                        be.dma_start(
                            bt[p0 + 1 : pe + 1, 1:2, :], t[p0:pe, R - 1 : R, :]
                        )
                        be.dma_start(
                            bt[p0 : p0 + 1, 1:2, :], t[p0 : p0 + 1, 0:1, :]
                        )

                    # ---- V pass: v[r] = min(row r-1, r, r+1) ----
                    # tmp[j] = min(row j, row j+1), j in [0, R-1); tmp[0] and
                    # tmp[R-2] double as the boundary rows' first min stage.
                    v = vt.tile([128, R, PW], F32)
                    tmp = vtm.tile([128, R - 1, PW], F32)
                    nc.vector.tensor_tensor(
                        out=tmp[0:P], in0=t[0:P, 0 : R - 1, :],
                        in1=t[0:P, 1:R, :], op=MIN,
                    )
                    nc.vector.tensor_tensor(
                        out=v[0:P, 1 : R - 1, :], in0=tmp[0:P, 0 : R - 2, :],
                        in1=t[0:P, 2:R, :], op=MIN,
                    )
                    nc.vector.tensor_tensor(
                        out=v[0:P, 0:1, :], in0=tmp[0:P, 0:1, :],
                        in1=bt[0:P, 1:2, :], op=MIN,
                    )
                    nc.vector.tensor_tensor(
                        out=v[0:P, R - 1 : R, :], in0=tmp[0:P, R - 2 : R - 1, :],
                        in1=bt[0:P, 0:1, :], op=MIN,
                    )

                    # ---- H pass: o[c] = min(v[c], v[c+1], v[c+2]) ----
                    h = htm.tile([128, R, cc + 1], F32)
                    nc.vector.tensor_tensor(
                        out=h[0:P], in0=v[0:P, :, 0 : cc + 1],
                        in1=v[0:P, :, 1 : cc + 2], op=MIN,
                    )
                    ot = outp.tile([128, R, cc], F32)
                    nc.vector.tensor_tensor(
                        out=ot[0:P], in0=h[0:P, :, 0:cc],
                        in1=v[0:P, :, 2 : cc + 2], op=MIN,
                    )

                    for i in range(n_img):
                        dst = o_d[s0 + i, :, c0 : c0 + cc].rearrange(
                            "(p r) w -> p r w", p=ppi
                        )
                        p0 = i * ppi
                        nc.scalar.dma_start(dst, ot[p0 : p0 + ppi, :, :])


def build_erosion(C, H, W, cc=CC, rpp=RPP, reps=1, bufs=None, beng="gpsimd"):
    """Per-core Bass program: x [C,H,W] f32 -> o [C,H,W] f32, erosion^reps."""
    assert H % rpp == 0
    ppi = H // rpp
    assert ppi <= 128 and W % cc == 0

    nc = bacc.Bacc("TRN2", target_bir_lowering=False, debug=False, num_devices=1)
    x_d = nc.dram_tensor("x", [C, H, W], F32, kind="ExternalInput").ap()
    o_d = nc.dram_tensor("o", [C, H, W], F32, kind="ExternalOutput").ap()
    # ping-pong DRAM scratch for chained passes
    s_d = [
        nc.dram_tensor(f"scratch{i}", [C, H, W], F32, kind="Internal").ap()
        for i in range(min(2, max(0, reps - 1)))
    ]

    def stage(i):
        # source/dest for pass i of reps
        src = x_d if i == 0 else s_d[(i - 1) % 2]
        dst = o_d if i == reps - 1 else s_d[i % 2]
        return src, dst

    bf = {"inp": 2, "bnd": 2, "vtm": 2, "vt": 2, "htm": 2, "outp": 2}
    if bufs:
        bf.update(bufs)
    with tile.TileContext(nc) as tc:
        with (
            tc.tile_pool(name="inp", bufs=bf["inp"]) as inp,
            tc.tile_pool(name="bnd", bufs=bf["bnd"]) as bnd,
            tc.tile_pool(name="vtm", bufs=bf["vtm"]) as vtm,
            tc.tile_pool(name="vt", bufs=bf["vt"]) as vt,
            tc.tile_pool(name="htm", bufs=bf["htm"]) as htm,
            tc.tile_pool(name="outp", bufs=bf["outp"]) as outp,
        ):
            pools = (inp, bnd, vtm, vt, htm, outp)
            for i in range(reps):
                src, dst = stage(i)
                _emit_pass(nc, pools, src, dst, C, H, W, cc, rpp, beng=beng)
    nc.compile()
    return nc


def _get_program(C, H, W, reps=1):
    key = (C, H, W, reps)
    if key not in _cache:
        _cache[key] = build_erosion(C, H, W, reps=reps)
    return _cache[key]


def kernel(x, m):
    from concourse.bass_utils import run_bass_kernel_spmd

    m = int(np.asarray(m))
    x = np.ascontiguousarray(np.asarray(x), dtype=np.float32)
    B, C, H, W = x.shape
    if m <= 0:
        return x.copy()
    # erosion by a (2m+1)-square = m chained 3x3 erosion passes in one NEFF
    nc = _get_program(C, H, W, reps=m)
    n_cores = 8
    assert B == n_cores, f"expected batch {n_cores}, got {B}"
    in_maps = [{"x": x[b]} for b in range(n_cores)]
    res = run_bass_kernel_spmd(nc, in_maps, core_ids=list(range(n_cores)))
    return np.stack([r["o"] for r in res.results], axis=0)


if __name__ == "__main__":
    # small-scale CoreSim correctness check (no hardware needed)
    from concourse.bass_interp import CoreSim

    rng = np.random.default_rng(0)
    C, H, W = 2, 128, 64
    x = rng.standard_normal((C, H, W)).astype(np.float32)
    nc = build_erosion(C, H, W, cc=32, rpp=16)
    sim = CoreSim(nc)
    sim.tensor("x")[:] = x
    sim.simulate(check_with_hw=False)
    got = sim.tensor("o")
    xp = np.pad(x, ((0, 0), (1, 1), (1, 1)), constant_values=PAD)
    exp = np.empty_like(x)
    for i in range(H):
        for j in range(W):
            exp[:, i, j] = xp[:, i : i + 3, j : j + 3].min(axis=(1, 2))
    ok = np.array_equal(got, exp)
    print("CoreSim small erosion ok:", ok)



# revision 9
# speedup vs baseline: 37178.8491x; 37178.8491x over previous
"""Trainium2 Bass kernel for 3x3 (k=2m+1) morphological erosion (sliding-window
min) over [B, C, H, W] fp32, B=8 sharded across 8 NeuronCores (one batch per
core).

v3 scheme (per core, shard = one batch of C=8 channel images, 1024x1024):
  - each partition holds RPP=16 consecutive image rows (ppi=64 partitions per
    image, 2 images per 128-partition stack), processed in CC=256-column
    chunks with a 1-column halo (PW=258).
  - the separable 3x3 min runs in bf16 on DVE at the 2x_1p rate (2-byte
    dtype + unit innermost stride). min never creates new values, so the
    total error is one bf16 rounding of the input (~2^-9 relative), far
    inside the 2e-2 gate.
  - the device kernel STORES bf16 (halves store traffic: 64MB -> 48MB/core
    round trip); kernel() widens to fp32 on the host after the gather.
  - ACT (scalar) engine does the fp32->bf16 input convert (first pass only;
    chained passes read bf16 scratch directly) and drives store DMAs; loads
    and the per-partition-block boundary-row halo copies ride the SP queue;
    PAD memsets go to Pool. Emission is software-pipelined (skew 2) so no
    in-order sequencer blocks a neighbor engine's next tile.
  - V pass: tmp[j] = min(row j, row j+1); interior v rows from tmp + row
    j+2; the 2 per-partition boundary rows take their missing neighbor from
    a small bf16 side tile bt (partition-shifted SBUF->SBUF DMA; image-edge
    partitions: PAD memset at block tops (legal start partitions 0/64),
    own-row duplication DMA at block bottoms).
  - Cost model: DMA engines ~147us, DVE ~141us, ACT ~55us -> balanced
    DMA/DVE at the bf16-store memory roofline.
  - m>1 runs as m chained passes (bf16 DRAM ping-pong) inside one NEFF.
"""

import sys

sys.path.insert(0, "/opt/trn_rl_repo")

import numpy as np

import concourse.bass as bass
import concourse.tile as tile
from concourse import bacc, mybir

PAD = 1.0e9
F32 = mybir.dt.float32
BF16 = mybir.dt.bfloat16
MIN = mybir.AluOpType.min

CC = 256  # column chunk width
RPP = 16  # image rows per partition

_cache = {}


def _emit_pass(nc, pools, x_d, o_d, C, H, W, cc, rpp, in_f32):
    """Emit one full erosion pass x_d -> o_d into the open TileContext.

    in_f32: x_d is fp32 and must be converted to bf16 on ACT; otherwise
    x_d is bf16 and is used directly. o_d is always bf16.
    """
    ppi = H // rpp  # partitions per image
    ips = max(1, 128 // ppi)  # images per partition-stack
    inp, xbp, bnd, vtm, vt, htm, obp = pools
    R = rpp
    PW = cc + 2  # padded tile width

    tiles = [(s0, c0) for s0 in range(0, C, ips) for c0 in range(0, W, cc)]
    front = {}

    def emit_front(i):
        s0, c0 = tiles[i]
        n_img = min(ips, C - s0)
        P = n_img * ppi
        last = c0 + cc == W
        t = inp.tile([128, R, PW], F32, name="t") if in_f32 else None
        xb = xbp.tile([128, R, PW], BF16, name="xb")
        ld = t if in_f32 else xb
        wlo = max(c0 - 1, 0)
        whi = min(c0 + cc + 1, W)
        dlo = 1 if c0 == 0 else 0
        for im in range(n_img):
            src = x_d[s0 + im, :, wlo:whi].rearrange("(p r) w -> p r w", p=ppi)
            p0 = im * ppi
            nc.sync.dma_start(ld[p0 : p0 + ppi, :, dlo : dlo + (whi - wlo)], src)
        if c0 == 0:
            nc.gpsimd.memset(ld[0:P, :, 0:1], PAD)
        if last:
            nc.gpsimd.memset(ld[0:P, :, PW - 1 : PW], PAD)
        if in_f32:
            nc.scalar.copy(xb[0:P], t[0:P])
        front[i] = (xb, n_img, P, s0, c0)

    def emit_back(i):
        xb, n_img, P, s0, c0 = front.pop(i)
        # boundary-row side tile (bf16): bt[p,0] = first row of the block
        # below (xb[p+1] row 0), bt[p,1] = last row of the block above
        # (xb[p-1] row R-1); image-edge partitions: top -> PAD memset
        # (start partition p0 is 0 mod ppi>=32: legal), bottom -> own-row
        # duplication DMA (min-idempotent; DMA has no start-partition rule).
        bt = bnd.tile([128, 2, PW], BF16)
        for im in range(n_img):
            p0 = im * ppi
            pe = p0 + ppi - 1  # last partition of this image
            nc.sync.dma_start(bt[p0:pe, 0:1, :], xb[p0 + 1 : pe + 1, 0:1, :])
            nc.sync.dma_start(
                bt[p0 + 1 : pe + 1, 1:2, :], xb[p0:pe, R - 1 : R, :]
            )
            nc.sync.dma_start(
                bt[pe : pe + 1, 0:1, :], xb[pe : pe + 1, R - 1 : R, :]
            )
            nc.gpsimd.memset(bt[p0 : p0 + 1, 1:2, :], PAD)

        # ---- V pass (bf16, 2x DVE): v[r] = min(row r-1, r, r+1) ----
        v = vt.tile([128, R, PW], BF16)
        tmp = vtm.tile([128, R - 1, PW], BF16)
        nc.vector.tensor_tensor(
            out=tmp[0:P], in0=xb[0:P, 0 : R - 1, :], in1=xb[0:P, 1:R, :], op=MIN
        )
        nc.vector.tensor_tensor(
            out=v[0:P, 1 : R - 1, :],
            in0=tmp[0:P, 0 : R - 2, :],
            in1=xb[0:P, 2:R, :],
            op=MIN,
        )
        nc.vector.tensor_tensor(
            out=v[0:P, 0:1, :], in0=tmp[0:P, 0:1, :], in1=bt[0:P, 1:2, :], op=MIN
        )
        nc.vector.tensor_tensor(
            out=v[0:P, R - 1 : R, :],
            in0=tmp[0:P, R - 2 : R - 1, :],
            in1=bt[0:P, 0:1, :],
            op=MIN,
        )

        # ---- H pass (bf16, 2x DVE): o[c] = min(v[c], v[c+1], v[c+2]) ----
        h = htm.tile([128, R, cc + 1], BF16)
        nc.vector.tensor_tensor(
            out=h[0:P], in0=v[0:P, :, 0 : cc + 1], in1=v[0:P, :, 1 : cc + 2],
            op=MIN,
        )
        ob = obp.tile([128, R, cc], BF16)
        nc.vector.tensor_tensor(
            out=ob[0:P], in0=h[0:P, :, 0:cc], in1=v[0:P, :, 2 : cc + 2], op=MIN
        )

        # store bf16 from ACT's queue (host widens to fp32 after gather)
        for im in range(n_img):
            dst = o_d[s0 + im, :, c0 : c0 + cc].rearrange(
                "(p r) w -> p r w", p=ppi
            )
            p0 = im * ppi
            nc.scalar.dma_start(dst, ob[p0 : p0 + ppi, :, :])

    # software-pipelined emission: tile i+skew's load/convert lands in
    # every queue before tile i's compute/store, so ACT's in-order
    # sequencer never delays DVE's next tile.
    skew = 2
    for i in range(len(tiles) + skew):
        if i < len(tiles):
            emit_front(i)
        if i >= skew:
            emit_back(i - skew)


def build_erosion(C, H, W, cc=CC, rpp=RPP, reps=1, bufs=None):
    """Per-core Bass program: x [C,H,W] f32 -> o [C,H,W] bf16, erosion^reps."""
    assert H % rpp == 0
    ppi = H // rpp
    assert ppi <= 128 and W % cc == 0

    nc = bacc.Bacc("TRN2", target_bir_lowering=False, debug=False, num_devices=1)
    x_d = nc.dram_tensor("x", [C, H, W], F32, kind="ExternalInput").ap()
    o_d = nc.dram_tensor("o", [C, H, W], BF16, kind="ExternalOutput").ap()
    # ping-pong DRAM scratch (bf16) for chained passes
    s_d = [
        nc.dram_tensor(f"scratch{i}", [C, H, W], BF16, kind="Internal").ap()
        for i in range(min(2, max(0, reps - 1)))
    ]

    def stage(i):
        src = x_d if i == 0 else s_d[(i - 1) % 2]
        dst = o_d if i == reps - 1 else s_d[i % 2]
        return src, dst

    bf = {"inp": 4, "xb": 3, "bnd": 2, "vtm": 1, "vt": 1, "htm": 1, "ob": 3}
    if bufs:
        bf.update(bufs)
    with tile.TileContext(nc) as tc:
        with (
            tc.tile_pool(name="inp", bufs=bf["inp"]) as inp,
            tc.tile_pool(name="xb", bufs=bf["xb"]) as xbp,
            tc.tile_pool(name="bnd", bufs=bf["bnd"]) as bnd,
            tc.tile_pool(name="vtm", bufs=bf["vtm"]) as vtm,
            tc.tile_pool(name="vt", bufs=bf["vt"]) as vt,
            tc.tile_pool(name="htm", bufs=bf["htm"]) as htm,
            tc.tile_pool(name="ob", bufs=bf["ob"]) as obp,
        ):
            pools = (inp, xbp, bnd, vtm, vt, htm, obp)
            for i in range(reps):
                src, dst = stage(i)
                _emit_pass(nc, pools, src, dst, C, H, W, cc, rpp, in_f32=(i == 0))
    nc.compile()
    return nc


def _get_program(C, H, W, reps=1):
    key = (C, H, W, reps)
    if key not in _cache:
        _cache[key] = build_erosion(C, H, W, reps=reps)
    return _cache[key]


def kernel(x, m):
    from concourse.bass_utils import run_bass_kernel_spmd

    m = int(np.asarray(m))
    x = np.ascontiguousarray(np.asarray(x), dtype=np.float32)
    B, C, H, W = x.shape
    if m <= 0:
        return x.copy()
    # erosion by a (2m+1)-square = m chained 3x3 erosion passes in one NEFF
    nc = _get_program(C, H, W, reps=m)
    n_cores = 8
    assert B == n_cores, f"expected batch {n_cores}, got {B}"
    in_maps = [{"x": x[b]} for b in range(n_cores)]
    res = run_bass_kernel_spmd(nc, in_maps, core_ids=list(range(n_cores)))
    # device output is bf16; widen to fp32 on the host
    return np.stack(
        [np.asarray(r["o"]).astype(np.float32) for r in res.results], axis=0
    )


if __name__ == "__main__":
    # small-scale CoreSim correctness check (no hardware needed)
    import ml_dtypes

    from concourse.bass_interp import CoreSim

    rng = np.random.default_rng(0)
    C, H, W = 2, 128, 64
    x = rng.standard_normal((C, H, W)).astype(np.float32)
    for reps in (1, 2):
        nc = build_erosion(C, H, W, cc=32, rpp=4, reps=reps)
        sim = CoreSim(nc)
        sim.tensor("x")[:] = x
        sim.simulate(check_with_hw=False)
        got = np.asarray(sim.tensor("o")).astype(np.float32)
        xr = x.astype(ml_dtypes.bfloat16).astype(np.float32)
        exp = xr
        for _ in range(reps):
            xp = np.pad(exp, ((0, 0), (1, 1), (1, 1)), constant_values=PAD)
            nxt = np.empty_like(exp)
            for i in range(H):
                for j in range(W):
                    nxt[:, i, j] = xp[:, i : i + 3, j : j + 3].min(axis=(1, 2))
            exp = nxt
        ok = np.array_equal(got, exp)
        rel = np.max(np.abs(got - exp) / np.maximum(np.abs(exp), 1e-6))
        print(f"CoreSim reps={reps} bf16-exact: {ok} rel={rel:.2e}")
